# revision 1
# baseline (speedup 1.0000x reference)
"""Trainium2 Bass kernel for a single llama-style transformer layer + output head.

Model (per reference):
    h  = rms_norm(x, ln1); q,k,v = proj(h); rope(q, k)
    attn (full, non-causal) per head; x += Wo @ ctx
    h2 = rms_norm(x, ln2); x += Wdown @ (silu(Wgate h2) * (Wup h2))
    logits = x @ W_out.T + b_out            -> reshape(-1, 8, 1024)

Sharding: 8 cores, data-parallel over (batch, seq-half): core c owns batch c//2,
sequence half c%2 (1024 query tokens). Each core computes K/V for its batch's
full 2048-token sequence (small duplicate work) so no collectives are needed.

On-chip convention: activations are FEATURE-MAJOR [d, t] so the contraction
dim of every matmul is the partition dim. Weights are passed pre-transposed
(and pre-tiled where needed) from the host, in bf16, with the rms-norm gains
folded in. PSUM accumulates in fp32; the residual stream stays fp32 in SBUF.
"""

import dataclasses
import math

import numpy as np
import ml_dtypes

import concourse.bass as bass
import concourse.bacc as bacc
import concourse.tile as tile
import concourse.mybir as mybir
from concourse import bass_utils
from concourse.alu_op_type import AluOpType

BF16 = mybir.dt.bfloat16
F32 = mybir.dt.float32
AF = mybir.ActivationFunctionType
NPBF = ml_dtypes.bfloat16

N_CORES = 8


@dataclasses.dataclass(frozen=True)
class Cfg:
    D: int = 1024      # model dim
    S: int = 2048      # full seq (per batch)
    TQ: int = 1024     # query tokens per core
    H: int = 16        # heads
    HD: int = 64       # head dim
    FF: int = 4096     # mlp intermediate
    V: int = 1024      # output head size
    NT: int = 512      # matmul moving-dim tile
    EPS: float = 1e-6
    THETA: float = 10000.0

    @property
    def CD(self):
        return self.D // 128

    @property
    def CF(self):
        return self.FF // 128

    @property
    def KT(self):
        return self.S // 128

    @property
    def TT(self):
        return self.TQ // 128

    @property
    def HPC(self):
        return 128 // self.HD  # heads per 128-partition chunk (2)


FULL = Cfg()
TINY = Cfg(D=256, S=256, TQ=128, H=4, HD=64, FF=512, V=256, NT=128)


def _nt_slices(total, nt):
    return [(i * nt, nt) for i in range(total // nt)]


def build_bass(cfg: Cfg, debug_outputs: bool = False):
    """Build the SPMD Bass program. Returns nc."""
    c = cfg
    nc = bacc.Bacc("TRN2", target_bir_lowering=False, debug=False,
                   num_devices=N_CORES)

    # register an eps const AP (activation() converts float biases to APs)
    _eps_t = nc.alloc_sbuf_tensor("const-eps", [128, 1], F32)
    nc.gpsimd.memset(_eps_t.ap(), c.EPS)
    nc.const_aps.aps[(F32, c.EPS)] = _eps_t.ap()

    dt = nc.dram_tensor
    x_fm = dt("x_fm", [c.D, c.S], BF16, kind="ExternalInput").ap()
    x_own = dt("x_own", [c.D, c.TQ], F32, kind="ExternalInput").ap()
    wqT = dt("wqT", [c.CD, 128, c.CD * 128], BF16, kind="ExternalInput").ap()
    wkT = dt("wkT", [c.CD, 128, c.CD * 128], BF16, kind="ExternalInput").ap()
    woT = dt("woT", [c.CD, 128, c.CD * 128], BF16, kind="ExternalInput").ap()
    wvT = dt("wvT", [c.D, c.D], BF16, kind="ExternalInput").ap()
    _W = min(512, c.FF)
    _n_fog = max(1, c.FF // 512)
    wgT = dt("wgT", [_n_fog, 128, c.CD * _W], BF16, kind="ExternalInput").ap()
    wuT = dt("wuT", [_n_fog, 128, c.CD * _W], BF16, kind="ExternalInput").ap()
    wdT = dt("wdT", [c.CD, 128, c.CF * 128], BF16, kind="ExternalInput").ap()
    woutT = dt("woutT", [c.D, c.V], BF16, kind="ExternalInput").ap()
    bias_row = dt("bias_row", [1, c.V], BF16, kind="ExternalInput").ap()
    cos_s = dt("cos_s", [128, c.S], BF16, kind="ExternalInput").ap()
    sin_s = dt("sin_s", [128, c.S], BF16, kind="ExternalInput").ap()
    cos_q = dt("cos_q", [128, c.TQ], BF16, kind="ExternalInput").ap()
    sin_q = dt("sin_q", [128, c.TQ], BF16, kind="ExternalInput").ap()
    shiftT = dt("shiftT", [128, 128], BF16, kind="ExternalInput").ap()
    sel = dt("sel", [c.H, c.D], BF16, kind="ExternalInput").ap()
    onesb_d = dt("onesb", [128, 128], BF16, kind="ExternalInput").ap()

    logits = dt("logits", [c.TQ, c.V], F32, kind="ExternalOutput").ap()
    dbg = {}
    if debug_outputs:
        for nm, shp in [("d_h", [c.D, c.S]), ("d_kr", [c.D, c.S]),
                        ("d_qr", [c.D, c.TQ]), ("d_ctxn", [c.D, c.TQ]),
                        ("d_xo2", [c.D, c.TQ]), ("d_h2", [c.D, c.TQ])]:
            dbg[nm] = dt(nm, shp, F32, kind="ExternalOutput").ap()

    with tile.TileContext(nc) as tc:
        # ---------- small whole-kernel constants ----------
        const = tc.alloc_tile_pool(name="const", bufs=1)
        ones_b = const.tile([128, 128], BF16)
        nc.sync.dma_start(ones_b[:], onesb_d[:])
        shift_sb = const.tile([128, 128], BF16)
        nc.sync.dma_start(shift_sb[:], shiftT[:])

        # ---------- right-side stack: long-lived cross-phase tensors ----------
        p_ctxn = tc.alloc_tile_pool(name="ctxn", bufs=1, side="right")
        ctxn = [p_ctxn.tile([128, c.TQ], BF16, name=f"ctxn{i}") for i in range(c.CD)]
        p_den = tc.alloc_tile_pool(name="den", bufs=1, side="right")
        den_sb = p_den.tile([c.H, c.TQ], F32)
        sel_sb = p_den.tile([c.H, c.D], BF16)
        nc.sync.dma_start(sel_sb[:], sel[:])
        bias_sb = p_den.tile([1, c.V], BF16)
        nc.sync.dma_start(bias_sb[:], bias_row[:])

        # ---------- left: K/V (+ Q later) outputs, span B -> C ----------
        p_kv = tc.alloc_tile_pool(name="kv", bufs=1)
        kr = [p_kv.tile([128, c.S], BF16, name=f"kr{i}") for i in range(c.CD)]
        # V token-major in fp8, one 128-wide group per head:
        # cols [0:HD) = V, col HD = ones (denominator trick), rest zero pad
        # (full-128 stationary width keeps the PE fast-weight-load path on)
        FP8 = mybir.dt.float8e4
        vt = [p_kv.tile([128, c.H * 128], FP8, name=f"vt{i}")
              for i in range(c.KT)]
        p_qr = tc.alloc_tile_pool(name="qr", bufs=1)
        qr = [p_qr.tile([128, c.TQ], BF16, name=f"qr{i}") for i in range(c.CD)]
        p_hq = tc.alloc_tile_pool(name="hq", bufs=1)
        hq = [p_hq.tile([128, c.TQ], BF16, name=f"hq{i}") for i in range(c.CD)]

        # =======================================================
        # PHASE A: rms1 stats (full seq + own half), h = x*rstd
        # =======================================================
        # B-phase weights/tables allocated early so their DMAs prefetch
        # during phase A's serial stats
        pB_w = tc.alloc_tile_pool(name="phB_w", bufs=2)
        cos_s_sb = pB_w.tile([128, c.S], BF16, name="cos_s_sb", bufs=1)
        nc.sync.dma_start(cos_s_sb[:], cos_s[:])
        sin_s_sb = pB_w.tile([128, c.S], BF16, name="sin_s_sb", bufs=1)
        nc.sync.dma_start(sin_s_sb[:], sin_s[:])

        pA_t = tc.alloc_tile_pool(name="phA_t", bufs=1)
        rstd = pA_t.tile([1, c.S], BF16, name="rstd")
        pA = tc.alloc_tile_pool(name="phA", bufs=1)
        h = [pA.tile([128, c.S], BF16, name=f"h{i}") for i in range(c.CD)]
        pA_s = tc.alloc_tile_pool(name="phA_s", bufs=2)

        pA_ss = tc.alloc_tile_pool(name="phA_ss", bufs=1, space="PSUM")
        ss = {o: pA_ss.tile([1, c.NT], F32, name=f"ss{o}")
              for (o, n) in _nt_slices(c.S, c.NT)}
        for cd in range(c.CD):
            for (o, n) in _nt_slices(c.S, c.NT):
                xt = pA_s.tile([128, c.NT], BF16, tag="xt")
                nc.sync.dma_start(xt[:], x_fm[cd * 128:(cd + 1) * 128, o:o + n])
                sq = pA_s.tile([128, c.NT], BF16, tag="sq", bufs=3)
                nc.vector.tensor_tensor(sq[:], xt[:], xt[:], op=AluOpType.mult)
                nc.tensor.matmul(ss[o][:], ones_b[:, 0:1], sq[:],
                                 start=(cd == 0), stop=(cd == c.CD - 1))
        # rsqrt(m) = exp(-0.5 * ln(m))
        for (o, n) in _nt_slices(c.S, c.NT):
            nc.scalar.activation(rstd[:, o:o + n], ss[o][:], AF.Ln,
                                 bias=c.EPS, scale=1.0 / c.D)
        nc.scalar.activation(rstd[:], rstd[:], AF.Exp, scale=-0.5)
        pA_ss.release()

        # broadcast rstd over partitions (PE outer product) -> bf16 SBUF
        p_rb = tc.alloc_tile_pool(name="p_rb", bufs=1)
        rb_sb = p_rb.tile([128, c.S], BF16, name="rb_sb")
        pA_rb = tc.alloc_tile_pool(name="phA_rb", bufs=2, space="PSUM")
        for (o, n) in _nt_slices(c.S, c.NT):
            rbt = pA_rb.tile([128, c.NT], F32, tag="rb")
            nc.tensor.matmul(rbt[:], ones_b[0:1, :], rstd[:, o:o + n],
                             start=True, stop=True)
            nc.vector.tensor_copy(rb_sb[:, o:o + n], rbt[:])
        pA_rb.release()
        for cd in range(c.CD):
            for (o, n) in _nt_slices(c.S, c.NT):
                xt = pA_s.tile([128, c.NT], BF16, tag="xt")
                nc.sync.dma_start(xt[:], x_fm[cd * 128:(cd + 1) * 128, o:o + n])
                nc.vector.tensor_tensor(h[cd][:, o:o + n], xt[:],
                                        rb_sb[:, o:o + n], op=AluOpType.mult)
        p_rb.release()
        pA_s.release()
        if debug_outputs:
            pdbg_a = tc.alloc_tile_pool(name="pdbg_a", bufs=2)
            for cd in range(c.CD):
                t = pdbg_a.tile([128, c.S], F32, tag="dbg")
                nc.vector.tensor_copy(t[:], h[cd][:])
                nc.sync.dma_start(dbg["d_h"][cd * 128:(cd + 1) * 128, :], t[:])
            pdbg_a.release()

        # =======================================================
        # PHASE B: K proj + rope, V proj (token-major), then Q
        # =======================================================
        pB_t = tc.alloc_tile_pool(name="phB_t", bufs=2)
        pB_ps = tc.alloc_tile_pool(name="phB_ps", bufs=4, space="PSUM")
        pB_ps2 = tc.alloc_tile_pool(name="phB_ps2", bufs=3, space="PSUM")

        def rope_combine(pool, raw, psk, cos_t, sin_t, off, n, dst):
            """dst = raw*cos + (S@raw)*sin, all [128, n] at abs offset off."""
            t1 = pool.tile([128, c.NT], BF16, tag="ropet1")
            nc.vector.tensor_tensor(t1[:, 0:n], raw[:], cos_t[:, off:off + n],
                                    op=AluOpType.mult)
            t2 = pool.tile([128, c.NT], BF16, tag="ropet2")
            nc.vector.tensor_tensor(t2[:, 0:n], psk[:], sin_t[:, off:off + n],
                                    op=AluOpType.mult)
            nc.vector.tensor_tensor(dst[:], t1[:, 0:n], t2[:, 0:n],
                                    op=AluOpType.add)

        # --- K projection + rope ---
        for mo in range(c.CD):
            wk_t = pB_w.tile([128, c.CD * 128], BF16, tag="wk")
            nc.sync.dma_start(
                wk_t[:], wkT[mo:mo + 1].rearrange("o p f -> (o p) f"))
            for (o, n) in _nt_slices(c.S, c.NT):
                pk = pB_ps.tile([128, c.NT], F32, tag="pproj")
                for kc in range(c.CD):
                    nc.tensor.matmul(pk[:], wk_t[:, kc * 128:(kc + 1) * 128],
                                     h[kc][:, o:o + n],
                                     start=(kc == 0), stop=(kc == c.CD - 1))
                raw = pB_t.tile([128, c.NT], BF16, tag="kraw")
                nc.scalar.copy(raw[:, 0:n], pk[:])
                psk = pB_ps2.tile([128, c.NT], F32, tag="pshift")
                nc.tensor.matmul(psk[:], shift_sb[:], raw[:, 0:n], start=True, stop=True)
                rope_combine(pB_t, raw[:, 0:n], psk[:], cos_s_sb, sin_s_sb, o, n,
                             kr[mo][:, o:o + n])

        # --- V projection (token-major, with ones column) ---
        TB = 8
        for tb in range(0, c.KT, TB):
            for (o, n) in _nt_slices(c.D, c.NT):
                wv_sl = pB_w.tile([128, c.CD * c.NT], BF16, tag="wv_sl", bufs=1)
                for kc in range(c.CD):
                    nc.sync.dma_start(wv_sl[:, kc * c.NT:(kc + 1) * c.NT],
                                      wvT[kc * 128:(kc + 1) * 128, o:o + n])
                for to in range(tb, min(tb + TB, c.KT)):
                    pv = pB_ps.tile([128, c.NT], F32, tag="pproj")
                    for kc in range(c.CD):
                        nc.tensor.matmul(pv[:], h[kc][:, to * 128:(to + 1) * 128],
                                         wv_sl[:, kc * c.NT: kc * c.NT + n],
                                         start=(kc == 0), stop=(kc == c.CD - 1))
                    nh = n // c.HD
                    h0 = o // c.HD
                    dstv = vt[to].rearrange("p (h e) -> p h e", e=128)
                    nc.vector.tensor_copy(
                        dstv[:, h0:h0 + nh, 0:c.HD],
                        pv.rearrange("p (h e) -> p h e", e=c.HD))
        for to in range(c.KT):
            dstv = vt[to].rearrange("p (h e) -> p h e", e=128)
            nc.gpsimd.memset(dstv[:, :, c.HD:c.HD + 1], 1.0)
            nc.gpsimd.memset(dstv[:, :, c.HD + 1:], 0.0)

        # own-half slice of h (the Q-side rms_norm equals the full-seq one)
        halves = c.S // c.TQ
        _pid = nc.vector.partition_id()
        qoff = (_pid % halves) * c.TQ
        for cd in range(c.CD):
            nc.vector.tensor_copy(hq[cd][:], h[cd][:, bass.ds(qoff, c.TQ)])

        if debug_outputs:
            pdbg_b = tc.alloc_tile_pool(name="pdbg_b", bufs=2)
            for cd in range(c.CD):
                t = pdbg_b.tile([128, c.S], F32, tag="dbgk")
                nc.vector.tensor_copy(t[:], kr[cd][:])
                nc.sync.dma_start(dbg["d_kr"][cd * 128:(cd + 1) * 128, :], t[:])
            pdbg_b.release()

        pB_ps2.release()
        pB_ps.release()
        pB_t.release()
        pA.release()       # h freed
        pA_t.release()
        pB_w.release()

        # --- Q: projection from hq + rope ---
        pQ = tc.alloc_tile_pool(name="phQ", bufs=1)
        cos_q_sb = pQ.tile([128, c.TQ], BF16, name="cos_q_sb")
        nc.sync.dma_start(cos_q_sb[:], cos_q[:])
        sin_q_sb = pQ.tile([128, c.TQ], BF16, name="sin_q_sb")
        nc.sync.dma_start(sin_q_sb[:], sin_q[:])
        pQ_t = tc.alloc_tile_pool(name="phQ_t", bufs=2)
        pQ_ps = tc.alloc_tile_pool(name="phQ_ps", bufs=3, space="PSUM")
        pQ_ps2 = tc.alloc_tile_pool(name="phQ_ps2", bufs=2, space="PSUM")

        for mo in range(c.CD):
            wq_t = pQ_t.tile([128, c.CD * 128], BF16, tag="wq")
            nc.sync.dma_start(
                wq_t[:], wqT[mo:mo + 1].rearrange("o p f -> (o p) f"))
            for (o, n) in _nt_slices(c.TQ, c.NT):
                pq = pQ_ps.tile([128, c.NT], F32, tag="pproj")
                for kc in range(c.CD):
                    nc.tensor.matmul(pq[:], wq_t[:, kc * 128:(kc + 1) * 128],
                                     hq[kc][:, o:o + n],
                                     start=(kc == 0), stop=(kc == c.CD - 1))
                raw = pQ_t.tile([128, c.NT], BF16, tag="qraw")
                nc.scalar.copy(raw[:, 0:n], pq[:])
                psk = pQ_ps2.tile([128, c.NT], F32, tag="pshiftq")
                nc.tensor.matmul(psk[:], shift_sb[:], raw[:, 0:n], start=True, stop=True)
                rope_combine(pQ_t, raw[:, 0:n], psk[:], cos_q_sb, sin_q_sb, o, n,
                             qr[mo][:, o:o + n])
        if debug_outputs:
            pdbg_q = tc.alloc_tile_pool(name="pdbg_q", bufs=2)
            for cd in range(c.CD):
                t2 = pdbg_q.tile([128, c.TQ], F32, tag="dbgq")
                nc.vector.tensor_copy(t2[:], qr[cd][:])
                nc.sync.dma_start(dbg["d_qr"][cd * 128:(cd + 1) * 128, :], t2[:])
            pdbg_q.release()
        pQ_ps2.release()
        pQ_ps.release()
        pQ_t.release()
        pQ.release()
        p_hq.release()

        # =======================================================
        # PHASE C: attention (ctxn holds unnormalized ctx, then
        # normalized in place)
        # =======================================================
        pC_exp = tc.alloc_tile_pool(name="phC_exp", bufs=1)
        pC_ps_sc = tc.alloc_tile_pool(name="phC_sc", bufs=2, space="PSUM")
        pC_ps_av = tc.alloc_tile_pool(name="phC_av", bufs=2, space="PSUM")

        def emit_scores_pair(hp):
            # two heads of one 128-partition chunk, kt-interleaved so the PE
            # alternates row halves (LDWEIGHTS of one overlaps the other)
            ch = hp
            exp_ab = ([], [])
            for kt in range(c.KT):
                for sl in range(c.HPC):
                    poff = sl * c.HD
                    sc = pC_ps_sc.tile([128, c.TQ], F32, tag=f"sc{sl}", bufs=1)
                    for (o, n) in _nt_slices(c.TQ, c.NT):
                        nc.tensor.matmul(
                            sc[:, o:o + n],
                            kr[ch][poff:poff + c.HD, kt * 128:(kt + 1) * 128],
                            qr[ch][poff:poff + c.HD, o:o + n],
                            start=True, stop=True)
                    e = pC_exp.tile([128, c.TQ], mybir.dt.float8e4,
                                    tag=f"exp{sl}_{kt}", bufs=2)
                    nc.scalar.activation(e[:], sc[:], AF.Exp)
                    exp_ab[sl].append(e)
            return exp_ab

        def emit_av_pair(hp, exp_ab):
            ch = hp
            for sl in range(c.HPC):
                hh = hp * c.HPC + sl
                poff = sl * c.HD
                exp_t = exp_ab[sl]
                for (o, n) in _nt_slices(c.TQ, c.NT):
                    av = pC_ps_av.tile([128, c.NT], F32, tag="av")
                    for kt in range(c.KT):
                        nc.tensor.matmul(av[:],
                                         vt[kt][:, hh * 128:(hh + 1) * 128],
                                         exp_t[kt][:, o:o + n],
                                         start=(kt == 0), stop=(kt == c.KT - 1))
                    nc.vector.tensor_copy(ctxn[ch][poff:poff + c.HD, o:o + n],
                                          av[0:c.HD, :])
                    # den row sits at psum partition HD(=64); engines cannot
                    # move it to partition hh: stage in SBUF, DMA-gather.
                    dstage = pC_exp.tile([128, c.NT], F32, tag="dstage", bufs=3)
                    nc.vector.tensor_copy(dstage[c.HD:c.HD + 1, :],
                                          av[c.HD:c.HD + 1, :])
                    nc.sync.dma_start(den_sb[hh:hh + 1, o:o + n],
                                      dstage[c.HD:c.HD + 1, :])

        # software-pipeline head pairs: scores(p+1) emitted before AV(p) so
        # the PE always has exp-independent matmul work while ACT runs exp
        prev = None
        for hp in range(c.H // c.HPC):
            et = emit_scores_pair(hp)
            if prev is not None:
                emit_av_pair(*prev)
            prev = (hp, et)
        emit_av_pair(*prev)

        # 1/x = exp(-ln(x));  recip lands in bf16 for the sel matmul
        nc.scalar.activation(den_sb[:], den_sb[:], AF.Ln)
        recip_bf = p_den.tile([c.H, c.TQ], BF16, name="recip_bf")
        nc.scalar.activation(recip_bf[:], den_sb[:], AF.Exp, scale=-1.0)
        pC_ps_av.release()
        pC_ps_sc.release()
        pC_rb = tc.alloc_tile_pool(name="phC_rb", bufs=2, space="PSUM")
        for ch in range(c.CD):
            for (o, n) in _nt_slices(c.TQ, c.NT):
                prb = pC_rb.tile([128, c.NT], F32, tag="prb")
                nc.tensor.matmul(prb[:], sel_sb[:, ch * 128:(ch + 1) * 128],
                                 recip_bf[:, o:o + n], start=True, stop=True)
                nc.vector.tensor_tensor(ctxn[ch][:, o:o + n], ctxn[ch][:, o:o + n],
                                        prb[:], op=AluOpType.mult)
        if debug_outputs:
            pdbg = tc.alloc_tile_pool(name="pdbgc", bufs=2)
            for cd in range(c.CD):
                t = pdbg.tile([128, c.TQ], F32, tag="dbgc")
                nc.vector.tensor_copy(t[:], ctxn[cd][:])
                nc.sync.dma_start(dbg["d_ctxn"][cd * 128:(cd + 1) * 128, :], t[:])
            pdbg.release()

        pC_rb.release()
        pC_exp.release()
        p_qr.release()
        p_kv.release()

        # =======================================================
        # PHASE D: Wo proj + residual, rms2, h2   (right-side pool)
        # =======================================================
        pD = tc.alloc_tile_pool(name="phD", bufs=1, side="right")
        xo2 = [pD.tile([128, c.TQ], F32, name=f"xo2_{i}") for i in range(c.CD)]
        h2 = [pD.tile([128, c.TQ], BF16, name=f"h2_{i}") for i in range(c.CD)]
        h3 = [pD.tile([128, c.TQ], BF16, name=f"h3_{i}") for i in range(c.CD)]
        pD_w = tc.alloc_tile_pool(name="phD_w", bufs=3)
        pD_t = tc.alloc_tile_pool(name="phD_t", bufs=3)
        pD_ps = tc.alloc_tile_pool(name="phD_ps", bufs=3, space="PSUM")
        pD_ps1 = tc.alloc_tile_pool(name="phD_ps1", bufs=1, space="PSUM")

        ss2 = {o: pD_ps1.tile([1, c.NT], F32, name=f"ss2_{o}")
               for (o, n) in _nt_slices(c.TQ, c.NT)}
        for mo in range(c.CD):
            wo_t = pD_w.tile([128, c.CD * 128], BF16, tag="wo")
            nc.sync.dma_start(
                wo_t[:], woT[mo:mo + 1].rearrange("o p f -> (o p) f"))
            xot = pD_t.tile([128, c.TQ], F32, tag="xot")
            nc.sync.dma_start(xot[:], x_own[mo * 128:(mo + 1) * 128, :])
            for (o, n) in _nt_slices(c.TQ, c.NT):
                po = pD_ps.tile([128, c.NT], F32, tag="po")
                for kc in range(c.CD):
                    nc.tensor.matmul(po[:], wo_t[:, kc * 128:(kc + 1) * 128],
                                     ctxn[kc][:, o:o + n],
                                     start=(kc == 0), stop=(kc == c.CD - 1))
                nc.vector.tensor_tensor(xo2[mo][:, o:o + n], xot[:, o:o + n],
                                        po[:], op=AluOpType.add)
                sq = pD_t.tile([128, c.NT], BF16, tag="sq2")
                nc.scalar.activation(sq[:], xo2[mo][:, o:o + n], AF.Square)
                nc.tensor.matmul(ss2[o][:], ones_b[:, 0:1], sq[:],
                                 start=(mo == 0), stop=(mo == c.CD - 1))
        rstd2 = pD_t.tile([1, c.TQ], BF16, tag="rstd2", bufs=1)
        for (o, n) in _nt_slices(c.TQ, c.NT):
            nc.scalar.activation(rstd2[:, o:o + n], ss2[o][:], AF.Ln,
                                 bias=c.EPS, scale=1.0 / c.D)
        nc.scalar.activation(rstd2[:], rstd2[:], AF.Exp, scale=-0.5)
        for (o, n) in _nt_slices(c.TQ, c.NT):
            rbt = pD_ps.tile([128, c.NT], F32, tag="po")
            nc.tensor.matmul(rbt[:], ones_b[0:1, :], rstd2[:, o:o + n],
                             start=True, stop=True)
            for cd in range(c.CD):
                nc.vector.tensor_tensor(h2[cd][:, o:o + n], xo2[cd][:, o:o + n],
                                        rbt[:], op=AluOpType.mult)

        if debug_outputs:
            for cd in range(c.CD):
                t = pD_t.tile([128, c.TQ], F32, tag="dbgd")
                nc.vector.tensor_copy(t[:], xo2[cd][:])
                nc.sync.dma_start(dbg["d_xo2"][cd * 128:(cd + 1) * 128, :], t[:])
                t2 = pD_t.tile([128, c.TQ], F32, tag="dbgd2")
                nc.vector.tensor_copy(t2[:], h2[cd][:])
                nc.sync.dma_start(dbg["d_h2"][cd * 128:(cd + 1) * 128, :], t2[:])

        pD_ps1.release()
        pD_ps.release()
        pD_t.release()
        pD_w.release()

        # =======================================================
        # PHASE E: MLP (swiglu), t-tile outer loop
        # =======================================================
        # output-head pools (head tiles are emitted inside the E loop per
        # t-half, as soon as that half's h3 chunks are complete)
        pF = tc.alloc_tile_pool(name="phF", bufs=1)
        wout_t = pF.tile([128, c.CD * c.V], BF16)
        for kc in range(c.CD):
            nc.sync.dma_start(wout_t[:, kc * c.V:(kc + 1) * c.V],
                              woutT[kc * 128:(kc + 1) * 128, :])
        pF_t = tc.alloc_tile_pool(name="phF_t", bufs=3)
        pF_ps = tc.alloc_tile_pool(name="phF_ps", bufs=2, space="PSUM")

        def emit_head(to):
            for (o, n) in _nt_slices(c.V, c.NT):
                ph = pF_ps.tile([128, c.NT], F32, tag="ph")
                for kc in range(c.CD):
                    nc.tensor.matmul(ph[:], h3[kc][:, to * 128:(to + 1) * 128],
                                     wout_t[:, kc * c.V + o: kc * c.V + o + n],
                                     start=(kc == 0), stop=False)
                nc.tensor.matmul(ph[:], ones_b[0:1, :], bias_sb[:, o:o + n],
                                 start=False, stop=True)
                lg = pF_t.tile([128, c.NT], F32, tag="lg")
                nc.vector.tensor_copy(lg[:], ph[:])
                nc.sync.dma_start(logits[to * 128:(to + 1) * 128, o:o + n], lg[:])

        FOG = 4 * 128  # gate/up weight slice width (columns of FF)
        pE = tc.alloc_tile_pool(name="phE", bufs=1)
        gu = [pE.tile([128, c.NT], BF16, name=f"gu{i}") for i in range(c.CF)]
        pE_w = tc.alloc_tile_pool(name="phE_w", bufs=2)
        pE_t = tc.alloc_tile_pool(name="phE_t", bufs=3)
        pE_ps = tc.alloc_tile_pool(name="phE_ps", bufs=2, space="PSUM")

        W = min(FOG, c.FF)
        n_fog = max(1, c.FF // FOG)
        fpg = c.CF // n_fog  # fo chunks per group
        for (o, n) in _nt_slices(c.TQ, c.NT):
            for fg in range(n_fog):
                wg_t = pE_w.tile([128, c.CD * W], BF16, tag="wg")
                wu_t = pE_w.tile([128, c.CD * W], BF16, tag="wu")
                nc.sync.dma_start(wg_t[:], wgT[fg:fg + 1].rearrange("o p f -> (o p) f"))
                nc.sync.dma_start(wu_t[:], wuT[fg:fg + 1].rearrange("o p f -> (o p) f"))
                for fi in range(fpg):
                    fo = fg * fpg + fi
                    pg = pE_ps.tile([128, c.NT], F32, tag="pg")
                    pu = pE_ps.tile([128, c.NT], F32, tag="pu")
                    for kc in range(c.CD):
                        nc.tensor.matmul(
                            pg[:, 0:n],
                            wg_t[:, kc * W + fi * 128: kc * W + (fi + 1) * 128],
                            h2[kc][:, o:o + n],
                            start=(kc == 0), stop=(kc == c.CD - 1))
                    for kc in range(c.CD):
                        nc.tensor.matmul(
                            pu[:, 0:n],
                            wu_t[:, kc * W + fi * 128: kc * W + (fi + 1) * 128],
                            h2[kc][:, o:o + n],
                            start=(kc == 0), stop=(kc == c.CD - 1))
                    g = pE_t.tile([128, c.NT], BF16, tag="g")
                    nc.scalar.activation(g[:, 0:n], pg[:, 0:n], AF.Silu)
                    nc.vector.tensor_tensor(gu[fo][:, 0:n], g[:, 0:n], pu[:, 0:n],
                                            op=AluOpType.mult)
            # down proj + residual -> h3 (bf16)
            for mo in range(c.CD):
                wd_t = pE_w.tile([128, c.CF * 128], BF16, tag="wd")
                nc.sync.dma_start(
                    wd_t[:], wdT[mo:mo + 1].rearrange("o p f -> (o p) f"))
                pd = pE_ps.tile([128, c.NT], F32, tag="pg")
                for fc in range(c.CF):
                    nc.tensor.matmul(pd[:, 0:n], wd_t[:, fc * 128:(fc + 1) * 128],
                                     gu[fc][:, 0:n],
                                     start=(fc == 0), stop=(fc == c.CF - 1))
                nc.vector.tensor_tensor(h3[mo][:, o:o + n], xo2[mo][:, o:o + n],
                                        pd[:, 0:n], op=AluOpType.add)
            for to in range(o // 128, (o + n) // 128):
                emit_head(to)

        pE_ps.release()
        pE_t.release()
        pE_w.release()
        pE.release()
        pF_ps.release()
        pF_t.release()
        pF.release()
        pD.release()
        p_den.release()
        p_ctxn.release()
        const.release()

    nc.compile()
    return nc


# ===================== host side =====================

def _bf(a):
    return np.ascontiguousarray(np.asarray(a, dtype=np.float32)).astype(NPBF)


def make_tables(c: Cfg):
    pos = np.arange(c.S, dtype=np.float32)
    inv = 1.0 / (c.THETA ** (np.arange(0, c.HD, 2, dtype=np.float32) / c.HD))
    ang = pos[:, None] * inv[None, :]                      # [S, HD/2]
    cos = np.concatenate([np.cos(ang), np.cos(ang)], -1).T  # [HD, S]
    sin = np.concatenate([np.sin(ang), np.sin(ang)], -1).T
    sign = np.where(np.arange(c.HD) < c.HD // 2, -1.0, 1.0)[:, None].astype(np.float32)
    cos_t = _bf(np.tile(cos, (c.HPC, 1)))                  # [128, S]
    sin_t = _bf(np.tile(sin * sign, (c.HPC, 1)))

    shiftT = np.zeros((128, 128), dtype=np.float32)
    for m in range(128):
        src = m + 32 if (m % c.HD) < c.HD // 2 else m - 32
        shiftT[src, m] = 1.0
    sel = np.zeros((c.H, c.D), dtype=np.float32)
    for ch in range(c.CD):
        for m in range(128):
            sel[ch * c.HPC + m // c.HD, ch * 128 + m] = 1.0
    return cos_t, sin_t, _bf(shiftT), _bf(sel)


def tile_lhsT(wT):
    """[K, M] -> [M/128 (mo), 128 (p), K (kc*128+f)] packed lhsT rows.

    out[mo, p, kc*128+f] = wT[kc*128+p, mo*128+f] so one contiguous DMA
    yields the SBUF tile whose [:, kc*128:(kc+1)*128] slice is the
    [K=128, M=128] stationary block for contraction chunk kc.
    """
    K, M = wT.shape
    t = wT.reshape(K // 128, 128, M // 128, 128)       # [kc, p, mo, f]
    return np.ascontiguousarray(t.transpose(2, 1, 0, 3).reshape(M // 128, 128, K))


NPF8 = ml_dtypes.float8_e4m3


def _f8(a):
    return np.asarray(a, dtype=np.float32).astype(NPF8)


def tile_fog(wT, W):
    """[D, FF] -> [FF/W (fg), 128 (p), (D/128)*W] packed gate/up slices."""
    D, FF = wT.shape
    t = wT.reshape(D // 128, 128, FF // W, W)          # [kc, p, fg, j]
    return np.ascontiguousarray(
        t.transpose(2, 1, 0, 3).reshape(FF // W, 128, D // 128 * W))


def prep_in_maps(c: Cfg, inputs: dict, n_cores: int = N_CORES):
    x = np.asarray(inputs["chunk_hidden_states"], dtype=np.float32)  # [B,S,D]
    ln1 = np.asarray(inputs["ln1_w"], dtype=np.float32)
    ln2 = np.asarray(inputs["ln2_w"], dtype=np.float32)
    wq = np.asarray(inputs["Wq"], dtype=np.float32)
    wk = np.asarray(inputs["Wk"], dtype=np.float32)
    wv = np.asarray(inputs["Wv"], dtype=np.float32)
    wo = np.asarray(inputs["Wo"], dtype=np.float32)
    wg = np.asarray(inputs["Wgate"], dtype=np.float32)
    wu = np.asarray(inputs["Wup"], dtype=np.float32)
    wd = np.asarray(inputs["Wdown"], dtype=np.float32)
    wout = np.asarray(inputs["W_out"], dtype=np.float32)
    b_out = np.asarray(inputs["b_out"], dtype=np.float32)

    W = min(512, c.FF)
    wqT = tile_lhsT(_bf((wq * ln1[None, :] / math.sqrt(c.HD)).T))
    wkT = tile_lhsT(_bf((wk * ln1[None, :]).T))
    woT = tile_lhsT(_bf(wo.T))
    wvT = _bf((wv * ln1[None, :]).T)
    wgT = tile_fog(_bf((wg * ln2[None, :]).T), W)
    wuT = tile_fog(_bf((wu * ln2[None, :]).T), W)
    wdT = tile_lhsT(_bf(wd.T))
    woutT = _bf(wout.T)
    bias_row = _bf(b_out[None, :])
    cos_t, sin_t, shiftT, sel = make_tables(c)
    onesb = np.ones((128, 128), dtype=np.float32).astype(NPBF)

    shared = dict(wqT=wqT, wkT=wkT, woT=woT, wvT=wvT, wgT=wgT, wuT=wuT,
                  wdT=wdT, woutT=woutT, bias_row=bias_row, cos_s=cos_t,
                  sin_s=sin_t, shiftT=shiftT, sel=sel, onesb=onesb)

    in_maps = []
    halves = c.S // c.TQ
    for core in range(n_cores):
        b, hf = core // halves, core % halves
        x_fm_f32 = np.ascontiguousarray(x[b].T)                  # [D, S]
        x_fm = x_fm_f32.astype(NPBF)
        x_own = np.ascontiguousarray(x_fm_f32[:, hf * c.TQ:(hf + 1) * c.TQ])
        m = dict(shared)
        m["x_fm"] = x_fm
        m["x_own"] = x_own
        m["cos_q"] = np.ascontiguousarray(cos_t[:, hf * c.TQ:(hf + 1) * c.TQ])
        m["sin_q"] = np.ascontiguousarray(sin_t[:, hf * c.TQ:(hf + 1) * c.TQ])
        in_maps.append(m)
    return in_maps


_NC_CACHE = {}


def _get_nc(cfg: Cfg):
    if cfg not in _NC_CACHE:
        _NC_CACHE[cfg] = build_bass(cfg)
    return _NC_CACHE[cfg]


def kernel(**inputs) -> np.ndarray:
    c = FULL
    nc = _get_nc(c)
    in_maps = prep_in_maps(c, inputs)
    res = bass_utils.run_bass_kernel_spmd(nc, in_maps, core_ids=list(range(N_CORES)))
    out = np.concatenate([res.results[i]["logits"] for i in range(N_CORES)], axis=0)
    return out.reshape(-1, 8, c.V)



# revision 8
# speedup vs baseline: 1.2577x; 1.2577x over previous
"""Trainium2 Bass kernel for a single llama-style transformer layer + output head.

Model (per reference):
    h  = rms_norm(x, ln1); q,k,v = proj(h); rope(q, k)
    attn (full, non-causal) per head; x += Wo @ ctx
    h2 = rms_norm(x, ln2); x += Wdown @ (silu(Wgate h2) * (Wup h2))
    logits = x @ W_out.T + b_out            -> reshape(-1, 8, 1024)

Sharding: 8 cores, data-parallel over (batch, seq-half): core c owns batch c//2,
sequence half c%2 (1024 query tokens). Each core computes K/V for its batch's
full 2048-token sequence (small duplicate work) so no collectives are needed.

On-chip convention: activations are FEATURE-MAJOR [d, t] so the contraction
dim of every matmul is the partition dim. Weights are passed pre-transposed
(and pre-tiled where needed) from the host, with the rms-norm gains folded in.
PSUM accumulates in fp32; the residual stream stays fp32 in SBUF.

fp8 use: attention probabilities and V are fp8 and the AV matmul runs in fp8
DoubleRow (two key-chunks per instruction); the MLP down-projection also runs
fp8 DoubleRow (gu scaled x32 via the up-weights, Wdown x64, unscaled in the
epilogue). Projection inner loops are weight-stationary: one LDWEIGHTS serves
every moving slice of that contraction chunk. Score matmuls for the two heads
of a 128-partition chunk are issued back-to-back on different PE row groups so
they run concurrently, and the previous head-pair's AV matmuls are interleaved
between key-chunks to fill exp-wait stalls.
"""

import dataclasses
import math

import numpy as np
import ml_dtypes

import concourse.bass as bass
import concourse.bacc as bacc
import concourse.tile as tile
import concourse.mybir as mybir
from concourse import bass_utils
from concourse.alu_op_type import AluOpType

BF16 = mybir.dt.bfloat16
F32 = mybir.dt.float32
FP8 = mybir.dt.float8e4
AF = mybir.ActivationFunctionType
DR = mybir.MatmulPerfMode.DoubleRow
NPBF = ml_dtypes.bfloat16
NPF8 = ml_dtypes.float8_e4m3

N_CORES = 8
GU_SCALE = 32.0
WD_SCALE = 64.0


@dataclasses.dataclass(frozen=True)
class Cfg:
    D: int = 1024      # model dim
    S: int = 2048      # full seq (per batch)
    TQ: int = 1024     # query tokens per core
    H: int = 16        # heads
    HD: int = 64       # head dim
    FF: int = 4096     # mlp intermediate
    V: int = 1024      # output head size
    NT: int = 512      # matmul moving-dim tile
    EPS: float = 1e-6
    THETA: float = 10000.0

    @property
    def CD(self):
        return self.D // 128

    @property
    def CF(self):
        return self.FF // 128

    @property
    def KT(self):
        return self.S // 128

    @property
    def HPC(self):
        return 128 // self.HD  # heads per 128-partition chunk (2)


FULL = Cfg()


def _nt_slices(total, nt):
    return [(i * nt, nt) for i in range(total // nt)]


def build_bass(cfg: Cfg):
    """Build the SPMD Bass program. Returns nc."""
    c = cfg
    nc = bacc.Bacc("TRN2", target_bir_lowering=False, debug=False,
                   num_devices=N_CORES)

    # register an eps const AP (activation() converts float biases to APs)
    _eps_t = nc.alloc_sbuf_tensor("const-eps", [128, 1], F32)
    nc.gpsimd.memset(_eps_t.ap(), c.EPS)
    nc.const_aps.aps[(F32, c.EPS)] = _eps_t.ap()

    dt = nc.dram_tensor
    x_fm = dt("x_fm", [c.D, c.S], BF16, kind="ExternalInput").ap()
    x_own = dt("x_own", [c.D, c.TQ], F32, kind="ExternalInput").ap()
    wqT = dt("wqT", [c.CD, 128, c.CD * 128], BF16, kind="ExternalInput").ap()
    wkT = dt("wkT", [c.CD, 128, c.CD * 128], BF16, kind="ExternalInput").ap()
    woT = dt("woT", [c.CD, 128, c.CD * 128], BF16, kind="ExternalInput").ap()
    wvT = dt("wvT", [c.D, c.D], BF16, kind="ExternalInput").ap()
    _W = min(512, c.FF)
    _n_fog = max(1, c.FF // 512)
    wgT = dt("wgT", [_n_fog, 128, c.CD * _W], BF16, kind="ExternalInput").ap()
    wuT = dt("wuT", [_n_fog, 128, c.CD * _W], BF16, kind="ExternalInput").ap()
    wdT = dt("wdT", [c.CD, 128, c.CF * 128], FP8, kind="ExternalInput").ap()
    woutT = dt("woutT", [c.D, c.V], BF16, kind="ExternalInput").ap()
    bias_row = dt("bias_row", [1, c.V], BF16, kind="ExternalInput").ap()
    cos_s = dt("cos_s", [128, c.S], BF16, kind="ExternalInput").ap()
    sin_s = dt("sin_s", [128, c.S], BF16, kind="ExternalInput").ap()
    shiftT = dt("shiftT", [128, 128], BF16, kind="ExternalInput").ap()
    sel = dt("sel", [c.H, c.D], BF16, kind="ExternalInput").ap()
    onesb_d = dt("onesb", [128, 128], BF16, kind="ExternalInput").ap()

    logits = dt("logits", [c.TQ, c.V], F32, kind="ExternalOutput").ap()

    with tile.TileContext(nc) as tc:
        # ---------- small whole-kernel constants ----------
        const = tc.alloc_tile_pool(name="const", bufs=1)
        ones_b = const.tile([128, 128], BF16)
        nc.sync.dma_start(ones_b[:], onesb_d[:])
        shift_sb = const.tile([128, 128], BF16)
        nc.sync.dma_start(shift_sb[:], shiftT[:])

        # ---------- right-side stack: long-lived cross-phase tensors ----------
        p_ctxn = tc.alloc_tile_pool(name="ctxn", bufs=1, side="right")
        ctxn = [p_ctxn.tile([128, c.TQ], BF16, name=f"ctxn{i}") for i in range(c.CD)]
        p_den = tc.alloc_tile_pool(name="den", bufs=1, side="right")
        den_sb = p_den.tile([c.H, c.TQ], F32)
        sel_sb = p_den.tile([c.H, c.D], BF16)
        nc.sync.dma_start(sel_sb[:], sel[:])
        bias_sb = p_den.tile([1, c.V], BF16)
        nc.sync.dma_start(bias_sb[:], bias_row[:])

        # ---------- left: K/V/Q outputs, span B -> C ----------
        p_kv = tc.alloc_tile_pool(name="kv", bufs=1)
        kr = [p_kv.tile([128, c.S], BF16, name=f"kr{i}") for i in range(c.CD)]
        # V token-major in fp8, paired key-chunks for DoubleRow AV:
        # vt2[tp][p, j, h*128+e] with j in {0,1} the key chunk 2*tp+j;
        # cols [0:HD) = V, col HD = ones (denominator trick), rest zero pad
        vt2 = [p_kv.tile([128, 2 * c.H * 128], FP8, name=f"vt2_{i}")
               for i in range(c.KT // 2)]
        p_qr = tc.alloc_tile_pool(name="qr", bufs=1)
        qr = [p_qr.tile([128, c.TQ], BF16, name=f"qr{i}") for i in range(c.CD)]

        # h lives A -> end of B (V projection consumes it)
        pA = tc.alloc_tile_pool(name="phA", bufs=1)
        h = [pA.tile([128, c.S], BF16, name=f"h{i}") for i in range(c.CD)]

        # rope tables: DMA'd up front; Q slices them at a dynamic offset
        pB_w = tc.alloc_tile_pool(name="phB_w", bufs=1)
        cos_s_sb = pB_w.tile([128, c.S], BF16, name="cos_s_sb")
        nc.sync.dma_start(cos_s_sb[:], cos_s[:])
        sin_s_sb = pB_w.tile([128, c.S], BF16, name="sin_s_sb")
        nc.sync.dma_start(sin_s_sb[:], sin_s[:])

        # =======================================================
        # PHASE A: rms1 stats over full seq; h = x*rstd  (x resident)
        # =======================================================
        pA_x = tc.alloc_tile_pool(name="phA_x", bufs=1)
        xk = [pA_x.tile([128, c.S], BF16, name=f"xk{i}") for i in range(c.CD)]
        for cd in range(c.CD):
            nc.sync.dma_start(xk[cd][:], x_fm[cd * 128:(cd + 1) * 128, :])

        pA_t = tc.alloc_tile_pool(name="phA_t", bufs=1)
        rstd = pA_t.tile([1, c.S], BF16, name="rstd")
        pA_s = tc.alloc_tile_pool(name="phA_s", bufs=3)
        pA_ss = tc.alloc_tile_pool(name="phA_ss", bufs=1, space="PSUM")
        ss = {o: pA_ss.tile([1, c.NT], F32, name=f"ss{o}")
              for (o, n) in _nt_slices(c.S, c.NT)}
        for cd in range(c.CD):
            for (o, n) in _nt_slices(c.S, c.NT):
                sq = pA_s.tile([128, c.NT], BF16, tag="sq")
                nc.vector.tensor_tensor(sq[:], xk[cd][:, o:o + n],
                                        xk[cd][:, o:o + n], op=AluOpType.mult)
                nc.tensor.matmul(ss[o][:], ones_b[:, 0:1], sq[:],
                                 start=(cd == 0), stop=(cd == c.CD - 1))
        # rsqrt(m) = exp(-0.5 * ln(m))
        for (o, n) in _nt_slices(c.S, c.NT):
            nc.scalar.activation(rstd[:, o:o + n], ss[o][:], AF.Ln,
                                 bias=c.EPS, scale=1.0 / c.D)
        nc.scalar.activation(rstd[:], rstd[:], AF.Exp, scale=-0.5)
        pA_ss.release()

        # broadcast rstd over partitions (PE outer product) -> bf16 SBUF
        p_rb = tc.alloc_tile_pool(name="p_rb", bufs=1)
        rb_sb = p_rb.tile([128, c.S], BF16, name="rb_sb")
        pA_rb = tc.alloc_tile_pool(name="phA_rb", bufs=2, space="PSUM")
        for (o, n) in _nt_slices(c.S, c.NT):
            rbt = pA_rb.tile([128, c.NT], F32, tag="rb")
            nc.tensor.matmul(rbt[:], ones_b[0:1, :], rstd[:, o:o + n],
                             start=True, stop=True)
            nc.vector.tensor_copy(rb_sb[:, o:o + n], rbt[:])
        pA_rb.release()
        for cd in range(c.CD):
            for (o, n) in _nt_slices(c.S, c.NT):
                nc.vector.tensor_tensor(h[cd][:, o:o + n], xk[cd][:, o:o + n],
                                        rb_sb[:, o:o + n], op=AluOpType.mult)
        p_rb.release()
        pA_s.release()
        pA_t.release()
        pA_x.release()

        # =======================================================
        # PHASE B: K proj + rope, V proj (token-major), then Q + rope
        # Weight-stationary inner loops: each LDWEIGHTS serves all
        # moving slices of that contraction chunk.
        # =======================================================
        pB_t = tc.alloc_tile_pool(name="phB_t", bufs=2)
        pB_wk = tc.alloc_tile_pool(name="phB_wk", bufs=2)
        pB_ps = tc.alloc_tile_pool(name="phB_ps", bufs=1, space="PSUM")
        pB_ps2 = tc.alloc_tile_pool(name="phB_ps2", bufs=2, space="PSUM")

        def rope_combine(pool, raw, psk, cos_ap, sin_ap, n, dst):
            """dst = raw*cos + (S@raw)*sin, all [128, n]."""
            t1 = pool.tile([128, c.NT], BF16, tag="ropet1")
            nc.vector.tensor_tensor(t1[:, 0:n], raw[:], cos_ap,
                                    op=AluOpType.mult)
            t2 = pool.tile([128, c.NT], BF16, tag="ropet2")
            nc.vector.tensor_tensor(t2[:, 0:n], psk[:], sin_ap,
                                    op=AluOpType.mult)
            nc.vector.tensor_tensor(dst[:], t1[:, 0:n], t2[:, 0:n],
                                    op=AluOpType.add)

        # --- K projection + rope (4 moving slices per stationary chunk) ---
        n_sl_s = c.S // c.NT
        for mo in range(c.CD):
            wk_t = pB_wk.tile([128, c.CD * 128], BF16, tag="wproj")
            nc.sync.dma_start(
                wk_t[:], wkT[mo:mo + 1].rearrange("o p f -> (o p) f"))
            pk = [pB_ps.tile([128, c.NT], F32, tag=f"pk{i}", name=f"pk{mo}_{i}")
                  for i in range(n_sl_s)]
            for kc in range(c.CD):
                for i, (o, n) in enumerate(_nt_slices(c.S, c.NT)):
                    nc.tensor.matmul(pk[i][:], wk_t[:, kc * 128:(kc + 1) * 128],
                                     h[kc][:, o:o + n],
                                     start=(kc == 0), stop=(kc == c.CD - 1))
            for i, (o, n) in enumerate(_nt_slices(c.S, c.NT)):
                raw = pB_t.tile([128, c.NT], BF16, tag="kraw")
                nc.scalar.copy(raw[:, 0:n], pk[i][:])
                psk = pB_ps2.tile([128, c.NT], F32, tag="pshift")
                nc.tensor.matmul(psk[:], shift_sb[:], raw[:, 0:n],
                                 start=True, stop=True)
                rope_combine(pB_t, raw[:, 0:n], psk[:],
                             cos_s_sb[:, o:o + n], sin_s_sb[:, o:o + n],
                             n, kr[mo][:, o:o + n])

        # --- V projection (token-major, fp8 pair layout) ---
        pV_w = tc.alloc_tile_pool(name="phV_w", bufs=1)
        wv_all = pV_w.tile([128, c.CD * c.D], BF16, name="wv_all")
        for kc in range(c.CD):
            nc.sync.dma_start(wv_all[:, kc * c.D:(kc + 1) * c.D],
                              wvT[kc * 128:(kc + 1) * 128, :])
        for to in range(c.KT):
            tp, j = to // 2, to % 2
            pv = [pB_ps.tile([128, c.NT], F32, tag=f"pk{i}", name=f"pv{to}_{i}")
                  for i in range(2)]
            for kc in range(c.CD):
                hsl = h[kc][:, to * 128:(to + 1) * 128]
                for o2 in range(2):
                    nc.tensor.matmul(pv[o2][:], hsl,
                                     wv_all[:, kc * c.D + o2 * c.NT:
                                            kc * c.D + (o2 + 1) * c.NT],
                                     start=(kc == 0), stop=(kc == c.CD - 1))
            v4 = vt2[tp].rearrange("p (j h e) -> p j h e", j=2, e=128)
            nh = c.NT // c.HD
            for o2 in range(2):
                nc.vector.tensor_copy(
                    v4[:, j, o2 * nh:(o2 + 1) * nh, 0:c.HD],
                    pv[o2].rearrange("p (h e) -> p h e", e=c.HD))
            nc.gpsimd.memset(v4[:, j, :, c.HD:c.HD + 1], 1.0)
            nc.gpsimd.memset(v4[:, j, :, c.HD + 1:], 0.0)
        pV_w.release()

        # own-half slice of h (the Q-side rms_norm equals the full-seq one)
        p_hq = tc.alloc_tile_pool(name="hq", bufs=1)
        hq = [p_hq.tile([128, c.TQ], BF16, name=f"hq{i}") for i in range(c.CD)]
        halves = c.S // c.TQ
        _pid = nc.vector.partition_id()
        qoff = (_pid % halves) * c.TQ
        for cd in range(c.CD):
            nc.vector.tensor_copy(hq[cd][:], h[cd][:, bass.ds(qoff, c.TQ)])

        # --- Q: projection from hq + rope (rope tables at dynamic offset) ---
        n_sl_q = c.TQ // c.NT
        for mo in range(c.CD):
            wq_t = pB_wk.tile([128, c.CD * 128], BF16, tag="wproj")
            nc.sync.dma_start(
                wq_t[:], wqT[mo:mo + 1].rearrange("o p f -> (o p) f"))
            pq = [pB_ps.tile([128, c.NT], F32, tag=f"pk{i}", name=f"pq{mo}_{i}")
                  for i in range(n_sl_q)]
            for kc in range(c.CD):
                for i, (o, n) in enumerate(_nt_slices(c.TQ, c.NT)):
                    nc.tensor.matmul(pq[i][:], wq_t[:, kc * 128:(kc + 1) * 128],
                                     hq[kc][:, o:o + n],
                                     start=(kc == 0), stop=(kc == c.CD - 1))
            for i, (o, n) in enumerate(_nt_slices(c.TQ, c.NT)):
                raw = pB_t.tile([128, c.NT], BF16, tag="qraw")
                nc.scalar.copy(raw[:, 0:n], pq[i][:])
                psk = pB_ps2.tile([128, c.NT], F32, tag="pshift")
                nc.tensor.matmul(psk[:], shift_sb[:], raw[:, 0:n],
                                 start=True, stop=True)
                rope_combine(pB_t, raw[:, 0:n], psk[:],
                             cos_s_sb[:, bass.ds(qoff + o, n)],
                             sin_s_sb[:, bass.ds(qoff + o, n)],
                             n, qr[mo][:, o:o + n])

        pB_ps2.release()
        pB_ps.release()
        p_hq.release()
        pB_wk.release()
        pB_t.release()
        pB_w.release()
        pA.release()       # h freed

        # =======================================================
        # PHASE C: attention. Per chunk (2 heads): scores + exp,
        # with the previous chunk's fp8 DoubleRow AV matmuls
        # interleaved per key-chunk to fill exp-wait stalls.
        # =======================================================
        pC_exp = tc.alloc_tile_pool(name="phC_exp", bufs=1)
        pC_ps_sc = tc.alloc_tile_pool(name="phC_sc", bufs=1, space="PSUM")
        pC_ps_av = tc.alloc_tile_pool(name="phC_av", bufs=2, space="PSUM")

        def av_jobs(hp, exp_t):
            """Yield thunks emitting one DoubleRow AV matmul each; the psum
            epilogue (ctx copy + den extraction) is emitted with the last."""
            ch = hp
            for sl in range(c.HPC):
                hh = hp * c.HPC + sl
                poff = sl * c.HD
                for (o, n) in _nt_slices(c.TQ, c.NT):
                    av = pC_ps_av.tile([128, c.NT], F32, tag="av")
                    for tp in range(c.KT // 2):
                        def mm(av=av, tp=tp, hh=hh, sl=sl, o=o, n=n,
                               poff=poff, ch=ch, exp_t=exp_t,
                               last=(tp == c.KT // 2 - 1)):
                            lhs = vt2[tp].rearrange(
                                "p (j f) -> p j f", j=2)[:, :, hh * 128:(hh + 1) * 128]
                            nc.tensor.matmul(av[:], lhs,
                                             exp_t[sl][tp][:, :, o:o + n],
                                             start=(tp == 0), stop=last,
                                             perf_mode=DR)
                            if last:
                                nc.vector.tensor_copy(
                                    ctxn[ch][poff:poff + c.HD, o:o + n],
                                    av[0:c.HD, :])
                                dstage = pC_exp.tile([128, c.NT], F32,
                                                     tag="dstage", bufs=3)
                                nc.vector.tensor_copy(dstage[c.HD:c.HD + 1, :],
                                                      av[c.HD:c.HD + 1, :])
                                nc.sync.dma_start(den_sb[hh:hh + 1, o:o + n],
                                                  dstage[c.HD:c.HD + 1, :])
                        yield mm

        def emit_scores(hp, prev_av):
            """Scores+exp for chunk hp; interleave prev chunk's AV matmuls."""
            ch = hp
            sc = [pC_ps_sc.tile([128, c.TQ], F32, tag=f"sc{sl}", name=f"sc{hp}_{sl}")
                  for sl in range(c.HPC)]
            exp_t = ([None] * (c.KT // 2), [None] * (c.KT // 2))
            for kt in range(c.KT):
                tp, j = kt // 2, kt % 2
                for (o, n) in _nt_slices(c.TQ, c.NT):
                    for sl in range(c.HPC):
                        poff = sl * c.HD
                        nc.tensor.matmul(
                            sc[sl][:, o:o + n],
                            kr[ch][poff:poff + c.HD, kt * 128:(kt + 1) * 128],
                            qr[ch][poff:poff + c.HD, o:o + n],
                            start=True, stop=True)
                for sl in range(c.HPC):
                    if exp_t[sl][tp] is None:
                        exp_t[sl][tp] = pC_exp.tile(
                            [128, 2, c.TQ], FP8, tag=f"exp{sl}_{tp}",
                            name=f"e{hp}_{sl}_{tp}", bufs=2)
                    nc.scalar.activation(exp_t[sl][tp][:, j, :], sc[sl][:], AF.Exp)
                # two AV matmuls of the previous pair between key-chunks
                if prev_av is not None:
                    for job in (next(prev_av, None), next(prev_av, None)):
                        if job is not None:
                            job()
            return exp_t

        prev = None
        for hp in range(c.H // c.HPC):
            prev_jobs = iter(av_jobs(*prev)) if prev is not None else None
            et = emit_scores(hp, prev_jobs)
            if prev_jobs is not None:
                for job in prev_jobs:
                    job()
            prev = (hp, et)
        for job in av_jobs(*prev):
            job()

        # 1/x = exp(-ln(x));  recip lands in bf16 for the sel matmul
        nc.scalar.activation(den_sb[:], den_sb[:], AF.Ln)
        recip_bf = p_den.tile([c.H, c.TQ], BF16, name="recip_bf")
        nc.scalar.activation(recip_bf[:], den_sb[:], AF.Exp, scale=-1.0)
        pC_ps_av.release()
        pC_ps_sc.release()
        pC_rb = tc.alloc_tile_pool(name="phC_rb", bufs=2, space="PSUM")
        for ch in range(c.CD):
            for (o, n) in _nt_slices(c.TQ, c.NT):
                prb = pC_rb.tile([128, c.NT], F32, tag="prb")
                nc.tensor.matmul(prb[:], sel_sb[:, ch * 128:(ch + 1) * 128],
                                 recip_bf[:, o:o + n], start=True, stop=True)
                nc.vector.tensor_tensor(ctxn[ch][:, o:o + n], ctxn[ch][:, o:o + n],
                                        prb[:], op=AluOpType.mult)
        pC_rb.release()
        pC_exp.release()
        p_qr.release()
        p_kv.release()

        # =======================================================
        # PHASE D: Wo proj + residual, rms2, h2   (right-side pool)
        # =======================================================
        pD = tc.alloc_tile_pool(name="phD", bufs=1, side="right")
        xo2 = [pD.tile([128, c.TQ], F32, name=f"xo2_{i}") for i in range(c.CD)]
        h2 = [pD.tile([128, c.TQ], BF16, name=f"h2_{i}") for i in range(c.CD)]
        h3 = [pD.tile([128, c.TQ], BF16, name=f"h3_{i}") for i in range(c.CD)]
        pD_w = tc.alloc_tile_pool(name="phD_w", bufs=2)
        pD_t = tc.alloc_tile_pool(name="phD_t", bufs=3)
        pD_ps = tc.alloc_tile_pool(name="phD_ps", bufs=1, space="PSUM")
        pD_ps2 = tc.alloc_tile_pool(name="phD_ps2", bufs=2, space="PSUM")
        pD_ps1 = tc.alloc_tile_pool(name="phD_ps1", bufs=1, space="PSUM")

        ss2 = {o: pD_ps1.tile([1, c.NT], F32, name=f"ss2_{o}")
               for (o, n) in _nt_slices(c.TQ, c.NT)}
        for mo in range(c.CD):
            wo_t = pD_w.tile([128, c.CD * 128], BF16, tag="wo")
            nc.sync.dma_start(
                wo_t[:], woT[mo:mo + 1].rearrange("o p f -> (o p) f"))
            po = [pD_ps.tile([128, c.NT], F32, tag=f"po{i}", name=f"po{mo}_{i}")
                  for i in range(n_sl_q)]
            for kc in range(c.CD):
                for i, (o, n) in enumerate(_nt_slices(c.TQ, c.NT)):
                    nc.tensor.matmul(po[i][:], wo_t[:, kc * 128:(kc + 1) * 128],
                                     ctxn[kc][:, o:o + n],
                                     start=(kc == 0), stop=(kc == c.CD - 1))
            xot = pD_t.tile([128, c.TQ], F32, tag="xot")
            nc.sync.dma_start(xot[:], x_own[mo * 128:(mo + 1) * 128, :])
            for i, (o, n) in enumerate(_nt_slices(c.TQ, c.NT)):
                nc.vector.tensor_tensor(xo2[mo][:, o:o + n], xot[:, o:o + n],
                                        po[i][:], op=AluOpType.add)
                sq = pD_t.tile([128, c.NT], BF16, tag="sq2")
                nc.scalar.activation(sq[:], xo2[mo][:, o:o + n], AF.Square)
                nc.tensor.matmul(ss2[o][:], ones_b[:, 0:1], sq[:],
                                 start=(mo == 0), stop=(mo == c.CD - 1))
        rstd2 = pD_t.tile([1, c.TQ], BF16, tag="rstd2", bufs=1)
        for (o, n) in _nt_slices(c.TQ, c.NT):
            nc.scalar.activation(rstd2[:, o:o + n], ss2[o][:], AF.Ln,
                                 bias=c.EPS, scale=1.0 / c.D)
        nc.scalar.activation(rstd2[:], rstd2[:], AF.Exp, scale=-0.5)
        for (o, n) in _nt_slices(c.TQ, c.NT):
            rbt = pD_ps2.tile([128, c.NT], F32, tag="rb2")
            nc.tensor.matmul(rbt[:], ones_b[0:1, :], rstd2[:, o:o + n],
                             start=True, stop=True)
            for cd in range(c.CD):
                nc.vector.tensor_tensor(h2[cd][:, o:o + n], xo2[cd][:, o:o + n],
                                        rbt[:], op=AluOpType.mult)

        pD_ps1.release()
        pD_ps2.release()
        pD_ps.release()
        pD_t.release()
        pD_w.release()

        # =======================================================
        # PHASE E: MLP (swiglu), t-tile outer loop
        # =======================================================
        # output-head pools (head tiles are emitted inside the E loop per
        # t-half, as soon as that half's h3 chunks are complete)
        pF = tc.alloc_tile_pool(name="phF", bufs=1)
        wout_t = pF.tile([128, c.CD * c.V], BF16)
        for kc in range(c.CD):
            nc.sync.dma_start(wout_t[:, kc * c.V:(kc + 1) * c.V],
                              woutT[kc * 128:(kc + 1) * 128, :])
        pF_t = tc.alloc_tile_pool(name="phF_t", bufs=3)
        pF_ps = tc.alloc_tile_pool(name="phF_ps", bufs=2, space="PSUM")

        def emit_head(to):
            for (o, n) in _nt_slices(c.V, c.NT):
                ph = pF_ps.tile([128, c.NT], F32, tag="ph")
                for kc in range(c.CD):
                    nc.tensor.matmul(ph[:], h3[kc][:, to * 128:(to + 1) * 128],
                                     wout_t[:, kc * c.V + o: kc * c.V + o + n],
                                     start=(kc == 0), stop=False)
                nc.tensor.matmul(ph[:], ones_b[0:1, :], bias_sb[:, o:o + n],
                                 start=False, stop=True)
                lg = pF_t.tile([128, c.NT], F32, tag="lg")
                nc.vector.tensor_copy(lg[:], ph[:])
                nc.sync.dma_start(logits[to * 128:(to + 1) * 128, o:o + n], lg[:])

        FOG = 4 * 128  # gate/up weight slice width (columns of FF)
        # gu in fp8 (x32 folded into up-weights) with fc-pairs adjacent so
        # the down projection runs fp8 DoubleRow.
        pE = tc.alloc_tile_pool(name="phE", bufs=1)
        guall = pE.tile([128, c.CF, c.NT], FP8, name="guall")
        pE_w = tc.alloc_tile_pool(name="phE_w", bufs=2)
        pE_t = tc.alloc_tile_pool(name="phE_t", bufs=3)
        pE_ps = tc.alloc_tile_pool(name="phE_ps", bufs=2, space="PSUM")

        W = min(FOG, c.FF)
        n_fog = max(1, c.FF // FOG)
        fpg = c.CF // n_fog  # fo chunks per group
        for (o, n) in _nt_slices(c.TQ, c.NT):
            for fg in range(n_fog):
                wg_t = pE_w.tile([128, c.CD * W], BF16, tag="wg")
                wu_t = pE_w.tile([128, c.CD * W], BF16, tag="wu")
                nc.sync.dma_start(wg_t[:], wgT[fg:fg + 1].rearrange("o p f -> (o p) f"))
                nc.sync.dma_start(wu_t[:], wuT[fg:fg + 1].rearrange("o p f -> (o p) f"))
                for fi in range(fpg):
                    fo = fg * fpg + fi
                    pg = pE_ps.tile([128, c.NT], F32, tag="pg")
                    pu = pE_ps.tile([128, c.NT], F32, tag="pu")
                    for kc in range(c.CD):
                        nc.tensor.matmul(
                            pg[:, 0:n],
                            wg_t[:, kc * W + fi * 128: kc * W + (fi + 1) * 128],
                            h2[kc][:, o:o + n],
                            start=(kc == 0), stop=(kc == c.CD - 1))
                    for kc in range(c.CD):
                        nc.tensor.matmul(
                            pu[:, 0:n],
                            wu_t[:, kc * W + fi * 128: kc * W + (fi + 1) * 128],
                            h2[kc][:, o:o + n],
                            start=(kc == 0), stop=(kc == c.CD - 1))
                    g = pE_t.tile([128, c.NT], BF16, tag="g")
                    nc.scalar.activation(g[:, 0:n], pg[:, 0:n], AF.Silu)
                    nc.vector.tensor_tensor(guall[:, fo, 0:n], g[:, 0:n],
                                            pu[:, 0:n], op=AluOpType.mult)
            # down proj (fp8 DoubleRow) + residual -> h3 (bf16)
            for mo in range(c.CD):
                wd_t = pE_w.tile([128, c.CF * 128], FP8, tag="wd")
                nc.sync.dma_start(
                    wd_t[:], wdT[mo:mo + 1].rearrange("o p f -> (o p) f"))
                wd_v = wd_t.rearrange("p (fc f) -> p fc f", f=128)
                pd = pE_ps.tile([128, c.NT], F32, tag="pg")
                for f2 in range(c.CF // 2):
                    nc.tensor.matmul(pd[:, 0:n],
                                     wd_v[:, 2 * f2:2 * f2 + 2, :],
                                     guall[:, 2 * f2:2 * f2 + 2, 0:n],
                                     start=(f2 == 0), stop=(f2 == c.CF // 2 - 1),
                                     perf_mode=DR)
                mt = pE_t.tile([128, c.NT], BF16, tag="mt")
                nc.scalar.mul(mt[:, 0:n], pd[:, 0:n], 1.0 / (GU_SCALE * WD_SCALE))
                nc.vector.tensor_tensor(h3[mo][:, o:o + n], xo2[mo][:, o:o + n],
                                        mt[:, 0:n], op=AluOpType.add)
            for to in range(o // 128, (o + n) // 128):
                emit_head(to)

        pE_ps.release()
        pE_t.release()
        pE_w.release()
        pE.release()
        pF_ps.release()
        pF_t.release()
        pF.release()
        pD.release()
        p_den.release()
        p_ctxn.release()
        const.release()

    nc.compile()
    return nc


# ===================== host side =====================

def _bf(a):
    return np.ascontiguousarray(np.asarray(a, dtype=np.float32)).astype(NPBF)


def make_tables(c: Cfg):
    pos = np.arange(c.S, dtype=np.float32)
    inv = 1.0 / (c.THETA ** (np.arange(0, c.HD, 2, dtype=np.float32) / c.HD))
    ang = pos[:, None] * inv[None, :]                      # [S, HD/2]
    cos = np.concatenate([np.cos(ang), np.cos(ang)], -1).T  # [HD, S]
    sin = np.concatenate([np.sin(ang), np.sin(ang)], -1).T
    sign = np.where(np.arange(c.HD) < c.HD // 2, -1.0, 1.0)[:, None].astype(np.float32)
    cos_t = _bf(np.tile(cos, (c.HPC, 1)))                  # [128, S]
    sin_t = _bf(np.tile(sin * sign, (c.HPC, 1)))

    shiftT = np.zeros((128, 128), dtype=np.float32)
    for m in range(128):
        src = m + 32 if (m % c.HD) < c.HD // 2 else m - 32
        shiftT[src, m] = 1.0
    sel = np.zeros((c.H, c.D), dtype=np.float32)
    for ch in range(c.CD):
        for m in range(128):
            sel[ch * c.HPC + m // c.HD, ch * 128 + m] = 1.0
    return cos_t, sin_t, _bf(shiftT), _bf(sel)


def tile_lhsT(wT):
    """[K, M] -> [M/128 (mo), 128 (p), K (kc*128+f)] packed lhsT rows.

    out[mo, p, kc*128+f] = wT[kc*128+p, mo*128+f] so one contiguous DMA
    yields the SBUF tile whose [:, kc*128:(kc+1)*128] slice is the
    [K=128, M=128] stationary block for contraction chunk kc.
    """
    K, M = wT.shape
    t = wT.reshape(K // 128, 128, M // 128, 128)       # [kc, p, mo, f]
    return np.ascontiguousarray(t.transpose(2, 1, 0, 3).reshape(M // 128, 128, K))


def tile_fog(wT, W):
    """[D, FF] -> [FF/W (fg), 128 (p), (D/128)*W] packed gate/up slices."""
    D, FF = wT.shape
    t = wT.reshape(D // 128, 128, FF // W, W)          # [kc, p, fg, j]
    return np.ascontiguousarray(
        t.transpose(2, 1, 0, 3).reshape(FF // W, 128, D // 128 * W))


def prep_in_maps(c: Cfg, inputs: dict, n_cores: int = N_CORES):
    x = np.asarray(inputs["chunk_hidden_states"], dtype=np.float32)  # [B,S,D]
    ln1 = np.asarray(inputs["ln1_w"], dtype=np.float32)
    ln2 = np.asarray(inputs["ln2_w"], dtype=np.float32)
    wq = np.asarray(inputs["Wq"], dtype=np.float32)
    wk = np.asarray(inputs["Wk"], dtype=np.float32)
    wv = np.asarray(inputs["Wv"], dtype=np.float32)
    wo = np.asarray(inputs["Wo"], dtype=np.float32)
    wg = np.asarray(inputs["Wgate"], dtype=np.float32)
    wu = np.asarray(inputs["Wup"], dtype=np.float32)
    wd = np.asarray(inputs["Wdown"], dtype=np.float32)
    wout = np.asarray(inputs["W_out"], dtype=np.float32)
    b_out = np.asarray(inputs["b_out"], dtype=np.float32)

    W = min(512, c.FF)
    wqT = tile_lhsT(_bf((wq * ln1[None, :] / math.sqrt(c.HD)).T))
    wkT = tile_lhsT(_bf((wk * ln1[None, :]).T))
    woT = tile_lhsT(_bf(wo.T))
    wvT = _bf((wv * ln1[None, :]).T)
    wgT = tile_fog(_bf((wg * ln2[None, :]).T), W)
    # x32 on the up-weights scales gu into fp8 range; x64 on Wdown keeps it
    # normal-range in fp8. The product 1/2048 is unscaled in the down epilogue.
    wuT = tile_fog(_bf((wu * ln2[None, :] * GU_SCALE).T), W)
    wdT = tile_lhsT((wd.T * WD_SCALE).astype(NPF8))
    woutT = _bf(wout.T)
    bias_row = _bf(b_out[None, :])
    cos_t, sin_t, shiftT, sel = make_tables(c)
    onesb = np.ones((128, 128), dtype=np.float32).astype(NPBF)

    shared = dict(wqT=wqT, wkT=wkT, woT=woT, wvT=wvT, wgT=wgT, wuT=wuT,
                  wdT=wdT, woutT=woutT, bias_row=bias_row, cos_s=cos_t,
                  sin_s=sin_t, shiftT=shiftT, sel=sel, onesb=onesb)

    in_maps = []
    halves = c.S // c.TQ
    for core in range(n_cores):
        b, hf = core // halves, core % halves
        x_fm_f32 = np.ascontiguousarray(x[b].T)                  # [D, S]
        x_fm = x_fm_f32.astype(NPBF)
        x_own = np.ascontiguousarray(x_fm_f32[:, hf * c.TQ:(hf + 1) * c.TQ])
        m = dict(shared)
        m["x_fm"] = x_fm
        m["x_own"] = x_own
        in_maps.append(m)
    return in_maps


_NC_CACHE = {}


def _get_nc(cfg: Cfg):
    if cfg not in _NC_CACHE:
        _NC_CACHE[cfg] = build_bass(cfg)
    return _NC_CACHE[cfg]


def kernel(**inputs) -> np.ndarray:
    c = FULL
    nc = _get_nc(c)
    in_maps = prep_in_maps(c, inputs)
    res = bass_utils.run_bass_kernel_spmd(nc, in_maps, core_ids=list(range(N_CORES)))
    out = np.concatenate([res.results[i]["logits"] for i in range(N_CORES)], axis=0)
    return out.reshape(-1, 8, c.V)
